# revision 30
# baseline (speedup 1.0000x reference)
"""Trainium2 Bass kernel for BatteryMoEFlattenIntraCycleMoELayer.

Computation (reference):
    gates = renorm(top2(softmax(logits) * mask))          # [B, E]
    x = cycle_curve_data.reshape(B, L, 900)
    out[b] = sum_e gates[b,e] * (x[b] @ W[e] + b[e])      # -> bf16 [B, L, 512]

Strategy (final; 336.6us baseline -> 206.6us measured on 8 trn2 cores):
  - Host: gates + top-2 routing (tiny). x transposed feat-major into
    [B, 128, 1024] bf16: cols 0..895 = chunks 0..6 (partition =
    feature-within-chunk), cols 896..1023 = tail rows 896..899 + a
    constant-1.0 bias row at partitions 0..4 (bias folds into the matmul
    via W_aug = [W; b]).
  - Host also PRE-MIXES the per-sample expert weights for contraction rows
    640..900 (chunks 5, 6 and the tail): mx[s] = g0*W_aug[e0] + g1*W_aug[e1]
    over those rows, divided by g0 (g0 >= 0.5 always after top-2 renorm) so
    the mixed matmuls can accumulate into the same PSUM bank as expert A
    (the combine multiplies it by g0). This turns 2 matmuls into 1 for
    those rows: per sample 13 instead of 16 N=512 streams, in exchange for
    +384 KB/sample of DMA - still under the PE roofline.
  - Shard B across 8 cores (64 samples each). One SPMD program: routing is
    data (per-sample W-slot offsets in PE registers for the shared chunks
    0..4; the mixed rows need no routing - they are per-sample tensors with
    static addresses). All 128 offsets are register-resident via 8
    TENSOR_LOADs tucked into the DMA-bound phase 1.
  - Dummy fp32 matmuls on the gates tile warm the PE HAM clock gate during
    the initial DMA wait (otherwise the first ~3.4us runs at 1.2 GHz).
  - Phase 1 (samples 0..7): DVE pre-scales x by the gates (xA=g0*x, xB=g1*x)
    so both experts accumulate into ONE PSUM bank per sample; k-outer over
    8 samples paces the PE against the streaming weight DMAs (banks cycle
    0..7 -> no same-bank back-to-back stall). Phase-1 samples get RAW mixed
    weights (no /g0) since their x is already gate-scaled. One ACT-or-DVE
    copy per sample.
  - Phase 2 (steady): pairs of samples; UNSCALED x stationary shared by the
    two expert matmuls of chunks 0..4 (one LDWEIGHTS per 2 matmuls)
    alternating two PSUM banks; the 3 mixed matmuls interleave across the
    pair so consecutive matmuls never hit the same PSUM bank. Combine =
    ACT mul (t=g1*psB) + DVE scalar_tensor_tensor (o = g0*psA + t, bf16),
    y out via ACT-issued DMA (keeps the Sync queue for x/w loads).
  - bf16 operands: same PE stream rate as f32r, half the DMA, fast FWL
    LDWEIGHTS the PE pulls ahead of in-flight matmuls.
"""

import os
import sys

for _p in ("/opt/trn_rl_repo", "/root/.axon_site/_ro/trn_rl_repo"):
    if os.path.isdir(_p) and _p not in sys.path:
        sys.path.insert(0, _p)

import numpy as np
import ml_dtypes

import concourse.bass as bass
import concourse.mybir as mybir
import concourse.tile as tile
from concourse import bacc
from concourse.bass_utils import run_bass_kernel_spmd
from concourse.bass_values import RuntimeValue

B, L, CURVE_LEN = 512, 128, 300
FEAT = 3 * CURVE_LEN          # 900
FEAT_AUG = FEAT + 1           # 901 (bias row)
D_MODEL = 512
NUM_EXPERTS = 8
TOP_K = 2
EPS = 1e-9
N_CORES = 8
S = B // N_CORES              # 64 samples per core
K_LAST = FEAT_AUG - 7 * 128   # 5
XCOLS = 1024                  # 8 x 128 cols (7 full chunks + padded tail)
WMAX = (NUM_EXPERTS - 1) * D_MODEL
P1 = 8                        # phase-1 sample count
N_SH = 5                      # shared (routed) chunks 0..4
N_MIX = 3                     # mixed blocks: chunks 5, 6, tail
MIX_ROW0 = N_SH * 128         # first mixed contraction row (640)
MXCOLS = N_MIX * D_MODEL      # 1536

_CACHE = {}


def _build_nc():
    """Build the SPMD Bass program (routing-independent)."""
    nc = bacc.Bacc(trn_type="TRN2")
    f32 = mybir.dt.float32
    bf16 = mybir.dt.bfloat16
    i32 = mybir.dt.int32

    x_h = nc.declare_dram_parameter("x", [S, 128, XCOLS], bf16, isOutput=False)
    w_h = nc.declare_dram_parameter("w", [N_SH, 128, NUM_EXPERTS, D_MODEL],
                                    bf16, isOutput=False)
    mx_h = nc.declare_dram_parameter("mx", [S, 128, MXCOLS], bf16,
                                     isOutput=False)
    g_h = nc.declare_dram_parameter("g", [128, 2 * S], f32, isOutput=False)
    widx_h = nc.declare_dram_parameter("widx", [1, 2 * S], i32, isOutput=False)
    y_h = nc.declare_dram_parameter("y", [S, L, D_MODEL], bf16, isOutput=True)

    with tile.TileContext(nc) as tc:
        with (
            tc.tile_pool(name="cpool", bufs=1) as cpool,
            tc.tile_pool(name="xpool", bufs=12) as xpool,
            tc.tile_pool(name="mxpool", bufs=12) as mxpool,
            tc.tile_pool(name="spool", bufs=8) as spool,
            tc.tile_pool(name="tpool", bufs=4) as tpool,
            tc.tile_pool(name="opool", bufs=4) as opool,
            tc.tile_pool(name="pspool", bufs=8, space="PSUM") as pspool,
        ):
            widx_sb = cpool.tile([1, 2 * S], i32)
            g_sb = cpool.tile([128, 2 * S], f32)
            w_sb = [cpool.tile([128, NUM_EXPERTS * D_MODEL], bf16,
                               name=f"w_sb_{k}") for k in range(N_SH)]

            def load_w(k):
                nc.sync.dma_start(
                    out=w_sb[k][:, :],
                    in_=w_h[k, :, :, :].rearrange("p e d -> p (e d)"),
                )

            # all 128 per-sample W-slot offsets stay register-resident
            wregs = [nc.tensor.alloc_register(f"widx_reg{i}")
                     for i in range(2 * S)]

            def load_widx_span(h):
                nc.tensor.reg_load(wregs[16 * h: 16 * h + 16],
                                   widx_sb[0:1, 16 * h: 16 * h + 16])

            def rv_of(s):
                a = RuntimeValue(val=wregs[2 * s], min_val=0, max_val=WMAX)
                b = RuntimeValue(val=wregs[2 * s + 1], min_val=0,
                                 max_val=WMAX)
                return a, b

            def load_x(s):
                x_sb = xpool.tile([128, XCOLS], bf16, tag="x",
                                  name=f"x_sb_{s}")
                nc.sync.dma_start(out=x_sb[:, :], in_=x_h[s, :, :])
                return x_sb

            def load_mx(s):
                mx_sb = mxpool.tile([128, MXCOLS], bf16, tag="mx",
                                    name=f"mx_sb_{s}")
                nc.sync.dma_start(out=mx_sb[:, :], in_=mx_h[s, :, :])
                return mx_sb

            def scale_x(s, x_sb):
                # phase-1 only: xs[:, 0:640] = g0 * x(shared chunks);
                # xs[:, 640:1280] = g1 * x(shared chunks)
                H = N_SH * 128
                xs = spool.tile([128, 2 * H], bf16, tag="xs",
                                name=f"xs_{s}")
                nc.vector.tensor_scalar_mul(
                    xs[:, 0: H], x_sb[:, 0: H], g_sb[:, 2 * s: 2 * s + 1])
                nc.vector.tensor_scalar_mul(
                    xs[:, H: 2 * H], x_sb[:, 0: H],
                    g_sb[:, 2 * s + 1: 2 * s + 2])
                return xs

            def mm_mix(ps, x_sb, mx_sb, c, stop):
                # mixed block c: x chunk N_SH+c vs per-sample mixed weights
                nc.tensor.matmul(
                    ps[:, :],
                    x_sb[:, (N_SH + c) * 128: (N_SH + c + 1) * 128],
                    mx_sb[:, c * D_MODEL: (c + 1) * D_MODEL],
                    start=False, stop=stop,
                )

            # ---------- DMA issue order == consumption order ----------
            nc.sync.dma_start(out=widx_sb[:, :], in_=widx_h[:, :])
            nc.sync.dma_start(out=g_sb[:, :], in_=g_h[:, :])
            load_w(0)
            p1_x = [load_x(s) for s in range(4)]
            load_w(1)
            p1_x += [load_x(s) for s in range(4, 8)]
            load_w(2)
            p1_mx = [load_mx(s) for s in range(2)]
            load_w(3)
            p1_mx += [load_mx(s) for s in range(2, 5)]
            load_w(4)
            p1_mx += [load_mx(s) for s in range(5, 8)]

            load_widx_span(0)
            # HAM warmup: ~3.4us of dummy fp32 matmuls on the gates tile
            ps_warm = pspool.tile([128, D_MODEL], f32, tag="ps",
                                  name="ps_warm")
            # 11 x ~427ns: warms the HAM clock gate AND delays the first
            # real matmul just enough that phase-1 never outruns the
            # streaming weight DMAs (PE pull-ahead caused idle windows
            # that sometimes re-throttled the clock mid-phase-1)
            for _ in range(11):
                nc.tensor.matmul(ps_warm[:, 0:128], g_sb[:, 0:128],
                                 g_sb[:, 0:128], start=True, stop=True)

            # ---------- phase 1: samples 0..7, scaled one-bank k-outer ----
            p1_xs = [scale_x(s, p1_x[s]) for s in range(P1)]
            p1_ps = [pspool.tile([128, D_MODEL], f32, tag="ps",
                                 name=f"ps_{s}") for s in range(P1)]
            p1_rv = [rv_of(s) for s in range(P1)]
            H = N_SH * 128
            for k in range(N_SH):
                for s in range(P1):       # expert-A sweep (banks cycle 0..7)
                    nc.tensor.matmul(
                        p1_ps[s][:, :],
                        p1_xs[s][:, k * 128: (k + 1) * 128],
                        w_sb[k][:, bass.ds(p1_rv[s][0], D_MODEL)],
                        start=(k == 0), stop=False,
                    )
                for s in range(P1):       # expert-B sweep
                    nc.tensor.matmul(
                        p1_ps[s][:, :],
                        p1_xs[s][:, H + k * 128: H + (k + 1) * 128],
                        w_sb[k][:, bass.ds(p1_rv[s][1], D_MODEL)],
                        start=False, stop=False,
                    )
                load_widx_span(k + 1)     # spans 1..5
            # mixed blocks sample-major (raw mix; stationary is the RAW x
            # tile) so consumption tracks the DMA arrival order of mx_s
            for s in range(P1):
                for c in range(N_MIX):
                    mm_mix(p1_ps[s], p1_x[s], p1_mx[s], c,
                           stop=(c == N_MIX - 1))
                if s in (0, 1):
                    load_widx_span(6 + s)  # spans 6, 7
            for s in range(P1):
                o_sb = opool.tile([128, D_MODEL], bf16, tag="o",
                                  name=f"o_{s}")
                # alternate engines so PSUM banks free 2-at-a-time
                if s % 2 == 0:
                    nc.scalar.copy(o_sb[:, :], p1_ps[s][:, :])
                else:
                    nc.vector.tensor_copy(o_sb[:, :], p1_ps[s][:, :])
                nc.scalar.dma_start(out=y_h[s, :, :], in_=o_sb[:, :])

            # ---------- phase 2: sample pairs, shared-stationary 2-bank ----
            for pair in range(P1 // 2, S // 2):
                s0 = 2 * pair
                xp, mxp, psA, psB, rvp = [], [], [], [], []
                for j in range(2):
                    s = s0 + j
                    xp.append(load_x(s))
                    mxp.append(load_mx(s))
                    psA.append(pspool.tile([128, D_MODEL], f32, tag="ps",
                                           name=f"psA_{s}"))
                    psB.append(pspool.tile([128, D_MODEL], f32, tag="ps",
                                           name=f"psB_{s}"))
                    rvp.append(rv_of(s))
                for j in range(2):
                    rvA, rvB = rvp[j]
                    for k in range(N_SH):
                        # shared stationary x chunk, banks alternate A/B
                        nc.tensor.matmul(
                            psA[j][:, :],
                            xp[j][:, k * 128: (k + 1) * 128],
                            w_sb[k][:, bass.ds(rvA, D_MODEL)],
                            start=(k == 0), stop=False,
                        )
                        nc.tensor.matmul(
                            psB[j][:, :],
                            xp[j][:, k * 128: (k + 1) * 128],
                            w_sb[k][:, bass.ds(rvB, D_MODEL)],
                            start=(k == 0), stop=(k == N_SH - 1),
                        )
                # mixed blocks interleave the pair -> banks alternate
                for c in range(N_MIX):
                    for j in range(2):
                        mm_mix(psA[j], xp[j], mxp[j], c,
                               stop=(c == N_MIX - 1))
                for j in range(2):
                    s = s0 + j
                    t1 = tpool.tile([128, D_MODEL], f32, tag="t",
                                    name=f"t1_{s}")
                    nc.scalar.mul(t1[:, :], psB[j][:, :],
                                  g_sb[:, 2 * s + 1: 2 * s + 2])
                    o_sb = opool.tile([128, D_MODEL], bf16, tag="o",
                                      name=f"o_{s}")
                    # o = gA * psA + t1  (one fused DVE pass)
                    nc.vector.scalar_tensor_tensor(
                        o_sb[:, :], psA[j][:, :],
                        g_sb[:, 2 * s: 2 * s + 1], t1[:, :],
                        mybir.AluOpType.mult, mybir.AluOpType.add,
                    )
                    nc.scalar.dma_start(out=y_h[s, :, :], in_=o_sb[:, :])

    nc.finalize()
    return nc


def _gates_np(logits, moe_masks):
    """Mirror reference _gates in numpy (fp32)."""
    lg = logits.astype(np.float32)
    m = lg.max(axis=1, keepdims=True)
    e = np.exp(lg - m)
    g = e / e.sum(axis=1, keepdims=True)
    g = g * (moe_masks == 1).astype(np.float32)
    # top-2, ties -> lower index first (matches jax.lax.top_k)
    top_idx = np.argsort(-g, axis=1, kind="stable")[:, :TOP_K]
    rows = np.arange(g.shape[0])[:, None]
    gsel = g[rows, top_idx]                                  # [B, 2]
    gsel = gsel / (gsel.sum(axis=1, keepdims=True) + EPS)
    return gsel.astype(np.float32), top_idx.astype(np.int32)


def _prep_inputs(cycle_curve_data, logits, moe_masks, W, b):
    gsel, top_idx = _gates_np(logits, moe_masks)

    xf = cycle_curve_data.reshape(B, L, FEAT).astype(np.float32, copy=False)
    # x_host[s, p, k*128 + l] = x[s, l, k*128 + p] for chunks 0..6;
    # tail rows 896..899 + bias row at partitions 0..4, rest zero
    x_host = np.zeros((B, 128, XCOLS), np.float32)
    x_host[:, :, : 7 * 128] = np.ascontiguousarray(
        xf[:, :, : 7 * 128].reshape(B, L, 7, 128).transpose(0, 3, 2, 1)
    ).reshape(B, 128, 7 * 128)
    x_host[:, :4, 7 * 128:] = xf[:, :, 7 * 128: FEAT].transpose(0, 2, 1)
    x_host[:, 4, 7 * 128:] = 1.0                             # bias row
    x_host = x_host.astype(ml_dtypes.bfloat16)

    w_aug = np.concatenate(
        [W.astype(np.float32), b.astype(np.float32)[:, None, :]], axis=1
    )                                                        # [E, 901, 512]
    w_host = np.zeros((N_SH, 128, NUM_EXPERTS, D_MODEL), np.float32)
    for k in range(N_SH):
        w_host[k] = w_aug[:, k * 128: (k + 1) * 128, :].transpose(1, 0, 2)
    w_host = w_host.astype(ml_dtypes.bfloat16)

    # per-sample mixed weights for rows 640..900 (chunks 5, 6, tail)
    mix = (gsel[:, 0, None, None] * w_aug[top_idx[:, 0], MIX_ROW0:, :]
           + gsel[:, 1, None, None] * w_aug[top_idx[:, 1], MIX_ROW0:, :])
    # phase-1 samples (one-bank, gate-scaled x) take the raw mix;
    # phase-2 samples take mix/g0 so it can ride the expert-A bank
    phase1 = (np.arange(B) % S) < P1
    inv = np.where(phase1, 1.0,
                   1.0 / np.maximum(gsel[:, 0], 1e-9)).astype(np.float32)
    mix *= inv[:, None, None]
    mx_host = np.zeros((B, 128, MXCOLS), np.float32)
    mx_host[:, :, 0: D_MODEL] = mix[:, 0:128]                # chunk 5
    mx_host[:, :, D_MODEL: 2 * D_MODEL] = mix[:, 128:256]    # chunk 6
    mx_host[:, :K_LAST, 2 * D_MODEL:] = mix[:, 256:261]      # tail + bias
    mx_host = mx_host.astype(ml_dtypes.bfloat16)

    in_maps = []
    for c in range(N_CORES):
        sl = slice(c * S, (c + 1) * S)
        g_rep = np.broadcast_to(
            gsel[sl].reshape(1, 2 * S), (128, 2 * S)
        ).copy()
        widx = (top_idx[sl].reshape(1, 2 * S) * D_MODEL).astype(np.int32)
        in_maps.append({
            "x": np.ascontiguousarray(x_host[sl]),
            "w": w_host,
            "mx": np.ascontiguousarray(mx_host[sl]),
            "g": g_rep,
            "widx": widx,
        })
    return in_maps


def kernel(cycle_curve_data, logits, moe_masks, W, b):
    if "nc" not in _CACHE:
        _CACHE["nc"] = _build_nc()
    nc = _CACHE["nc"]

    in_maps = _prep_inputs(cycle_curve_data, logits, moe_masks, W, b)

    trace = bool(int(os.environ.get("KERNEL_PROFILE", "0")))
    res = run_bass_kernel_spmd(
        nc, in_maps, core_ids=list(range(N_CORES)), trace=trace
    )
    _CACHE["last_results"] = res

    out = np.empty((B, L, D_MODEL), ml_dtypes.bfloat16)
    for c in range(N_CORES):
        out[c * S: (c + 1) * S] = res.results[c]["y"]
    return out


# revision 32
# speedup vs baseline: 1.0173x; 1.0173x over previous
"""Trainium2 Bass kernel for BatteryMoEFlattenIntraCycleMoELayer.

Computation (reference):
    gates = renorm(top2(softmax(logits) * mask))          # [B, E]
    x = cycle_curve_data.reshape(B, L, 900)
    out[b] = sum_e gates[b,e] * (x[b] @ W[e] + b[e])      # -> bf16 [B, L, 512]

Strategy (final; 336.6us baseline -> 206.6us measured on 8 trn2 cores):
  - Host: gates + top-2 routing (tiny). x transposed feat-major into
    [B, 128, 1024] bf16: cols 0..895 = chunks 0..6 (partition =
    feature-within-chunk), cols 896..1023 = tail rows 896..899 + a
    constant-1.0 bias row at partitions 0..4 (bias folds into the matmul
    via W_aug = [W; b]).
  - Host also PRE-MIXES the per-sample expert weights for contraction rows
    640..900 (chunks 5, 6 and the tail): mx[s] = g0*W_aug[e0] + g1*W_aug[e1]
    over those rows, divided by g0 (g0 >= 0.5 always after top-2 renorm) so
    the mixed matmuls can accumulate into the same PSUM bank as expert A
    (the combine multiplies it by g0). This turns 2 matmuls into 1 for
    those rows: per sample 13 instead of 16 N=512 streams, in exchange for
    +384 KB/sample of DMA - still under the PE roofline.
  - Shard B across 8 cores (64 samples each). One SPMD program: routing is
    data (per-sample W-slot offsets in PE registers for the shared chunks
    0..4; the mixed rows need no routing - they are per-sample tensors with
    static addresses). All 128 offsets are register-resident via 8
    TENSOR_LOADs tucked into the DMA-bound phase 1.
  - Dummy fp32 matmuls on the gates tile warm the PE HAM clock gate during
    the initial DMA wait (otherwise the first ~3.4us runs at 1.2 GHz).
  - Phase 1 (samples 0..7): DVE pre-scales x by the gates (xA=g0*x, xB=g1*x)
    so both experts accumulate into ONE PSUM bank per sample; k-outer over
    8 samples paces the PE against the streaming weight DMAs (banks cycle
    0..7 -> no same-bank back-to-back stall). Phase-1 samples get RAW mixed
    weights (no /g0) since their x is already gate-scaled. One ACT-or-DVE
    copy per sample.
  - Phase 2 (steady): pairs of samples; UNSCALED x stationary shared by the
    two expert matmuls of chunks 0..4 (one LDWEIGHTS per 2 matmuls)
    alternating two PSUM banks; the 3 mixed matmuls interleave across the
    pair so consecutive matmuls never hit the same PSUM bank. Combine =
    ACT mul (t=g1*psB) + DVE scalar_tensor_tensor (o = g0*psA + t, bf16),
    y out via ACT-issued DMA (keeps the Sync queue for x/w loads).
  - bf16 operands: same PE stream rate as f32r, half the DMA, fast FWL
    LDWEIGHTS the PE pulls ahead of in-flight matmuls.
"""

import os
import sys

for _p in ("/opt/trn_rl_repo", "/root/.axon_site/_ro/trn_rl_repo"):
    if os.path.isdir(_p) and _p not in sys.path:
        sys.path.insert(0, _p)

import numpy as np
import ml_dtypes

import concourse.bass as bass
import concourse.mybir as mybir
import concourse.tile as tile
from concourse import bacc
from concourse.bass_utils import run_bass_kernel_spmd
from concourse.bass_values import RuntimeValue

B, L, CURVE_LEN = 512, 128, 300
FEAT = 3 * CURVE_LEN          # 900
FEAT_AUG = FEAT + 1           # 901 (bias row)
D_MODEL = 512
NUM_EXPERTS = 8
TOP_K = 2
EPS = 1e-9
N_CORES = 8
S = B // N_CORES              # 64 samples per core
K_LAST = FEAT_AUG - 7 * 128   # 5
XCOLS = 1024                  # 8 x 128 cols (7 full chunks + padded tail)
WMAX = (NUM_EXPERTS - 1) * D_MODEL
P1 = 8                        # phase-1 sample count
N_SH = 5                      # shared (routed) chunks 0..4
N_MIX = 3                     # mixed blocks: chunks 5, 6, tail
MIX_ROW0 = N_SH * 128         # first mixed contraction row (640)
MXCOLS = N_MIX * D_MODEL      # 1536

_CACHE = {}


def _build_nc():
    """Build the SPMD Bass program (routing-independent)."""
    nc = bacc.Bacc(trn_type="TRN2")
    f32 = mybir.dt.float32
    bf16 = mybir.dt.bfloat16
    i32 = mybir.dt.int32

    x_h = nc.declare_dram_parameter("x", [S, 128, XCOLS], bf16, isOutput=False)
    w_h = nc.declare_dram_parameter("w", [N_SH, 128, NUM_EXPERTS, D_MODEL],
                                    bf16, isOutput=False)
    mx_h = nc.declare_dram_parameter("mx", [S, 128, MXCOLS], bf16,
                                     isOutput=False)
    g_h = nc.declare_dram_parameter("g", [128, 2 * S], f32, isOutput=False)
    widx_h = nc.declare_dram_parameter("widx", [1, 2 * S], i32, isOutput=False)
    y_h = nc.declare_dram_parameter("y", [S, L, D_MODEL], bf16, isOutput=True)

    with tile.TileContext(nc) as tc:
        with (
            tc.tile_pool(name="cpool", bufs=1) as cpool,
            tc.tile_pool(name="xpool", bufs=14) as xpool,
            tc.tile_pool(name="mxpool", bufs=14) as mxpool,
            tc.tile_pool(name="spool", bufs=8) as spool,
            tc.tile_pool(name="tpool", bufs=6) as tpool,
            tc.tile_pool(name="opool", bufs=6) as opool,
            tc.tile_pool(name="pspool", bufs=8, space="PSUM") as pspool,
        ):
            widx_sb = cpool.tile([1, 2 * S], i32)
            g_sb = cpool.tile([128, 2 * S], f32)
            w_sb = [cpool.tile([128, NUM_EXPERTS * D_MODEL], bf16,
                               name=f"w_sb_{k}") for k in range(N_SH)]

            def load_w(k):
                nc.sync.dma_start(
                    out=w_sb[k][:, :],
                    in_=w_h[k, :, :, :].rearrange("p e d -> p (e d)"),
                )

            # all 128 per-sample W-slot offsets stay register-resident
            wregs = [nc.tensor.alloc_register(f"widx_reg{i}")
                     for i in range(2 * S)]

            def load_widx_span(h):
                nc.tensor.reg_load(wregs[16 * h: 16 * h + 16],
                                   widx_sb[0:1, 16 * h: 16 * h + 16])

            def rv_of(s):
                a = RuntimeValue(val=wregs[2 * s], min_val=0, max_val=WMAX)
                b = RuntimeValue(val=wregs[2 * s + 1], min_val=0,
                                 max_val=WMAX)
                return a, b

            def load_x(s):
                x_sb = xpool.tile([128, XCOLS], bf16, tag="x",
                                  name=f"x_sb_{s}")
                nc.sync.dma_start(out=x_sb[:, :], in_=x_h[s, :, :])
                return x_sb

            def load_mx(s):
                mx_sb = mxpool.tile([128, MXCOLS], bf16, tag="mx",
                                    name=f"mx_sb_{s}")
                nc.sync.dma_start(out=mx_sb[:, :], in_=mx_h[s, :, :])
                return mx_sb

            def scale_x(s, x_sb):
                # phase-1 only: xs[:, 0:640] = g0 * x(shared chunks);
                # xs[:, 640:1280] = g1 * x(shared chunks)
                H = N_SH * 128
                xs = spool.tile([128, 2 * H], bf16, tag="xs",
                                name=f"xs_{s}")
                nc.vector.tensor_scalar_mul(
                    xs[:, 0: H], x_sb[:, 0: H], g_sb[:, 2 * s: 2 * s + 1])
                nc.vector.tensor_scalar_mul(
                    xs[:, H: 2 * H], x_sb[:, 0: H],
                    g_sb[:, 2 * s + 1: 2 * s + 2])
                return xs

            def mm_mix(ps, x_sb, mx_sb, c, stop):
                # mixed block c: x chunk N_SH+c vs per-sample mixed weights
                nc.tensor.matmul(
                    ps[:, :],
                    x_sb[:, (N_SH + c) * 128: (N_SH + c + 1) * 128],
                    mx_sb[:, c * D_MODEL: (c + 1) * D_MODEL],
                    start=False, stop=stop,
                )

            # ---------- DMA issue order == consumption order ----------
            nc.sync.dma_start(out=widx_sb[:, :], in_=widx_h[:, :])
            nc.sync.dma_start(out=g_sb[:, :], in_=g_h[:, :])
            load_w(0)
            p1_x = [load_x(s) for s in range(4)]
            load_w(1)
            p1_x += [load_x(s) for s in range(4, 8)]
            load_w(2)
            p1_mx = [load_mx(s) for s in range(2)]
            load_w(3)
            p1_mx += [load_mx(s) for s in range(2, 5)]
            load_w(4)
            p1_mx += [load_mx(s) for s in range(5, 8)]

            load_widx_span(0)
            # HAM warmup: ~3.4us of dummy fp32 matmuls on the gates tile
            ps_warm = pspool.tile([128, D_MODEL], f32, tag="ps",
                                  name="ps_warm")
            for _ in range(8):
                nc.tensor.matmul(ps_warm[:, 0:128], g_sb[:, 0:128],
                                 g_sb[:, 0:128], start=True, stop=True)

            # ---------- phase 1: samples 0..7, scaled one-bank k-outer ----
            p1_xs = [scale_x(s, p1_x[s]) for s in range(P1)]
            p1_ps = [pspool.tile([128, D_MODEL], f32, tag="ps",
                                 name=f"ps_{s}") for s in range(P1)]
            p1_rv = [rv_of(s) for s in range(P1)]
            H = N_SH * 128
            for k in range(N_SH):
                for s in range(P1):       # expert-A sweep (banks cycle 0..7)
                    nc.tensor.matmul(
                        p1_ps[s][:, :],
                        p1_xs[s][:, k * 128: (k + 1) * 128],
                        w_sb[k][:, bass.ds(p1_rv[s][0], D_MODEL)],
                        start=(k == 0), stop=False,
                    )
                for s in range(P1):       # expert-B sweep
                    nc.tensor.matmul(
                        p1_ps[s][:, :],
                        p1_xs[s][:, H + k * 128: H + (k + 1) * 128],
                        w_sb[k][:, bass.ds(p1_rv[s][1], D_MODEL)],
                        start=False, stop=False,
                    )
                load_widx_span(k + 1)     # spans 1..5
            # mixed blocks sample-major (raw mix; stationary is the RAW x
            # tile) so consumption tracks the DMA arrival order of mx_s
            for s in range(P1):
                for c in range(N_MIX):
                    mm_mix(p1_ps[s], p1_x[s], p1_mx[s], c,
                           stop=(c == N_MIX - 1))
                if s in (0, 1):
                    load_widx_span(6 + s)  # spans 6, 7
            for s in range(P1):
                o_sb = opool.tile([128, D_MODEL], bf16, tag="o",
                                  name=f"o_{s}")
                # alternate engines so PSUM banks free 2-at-a-time
                if s % 2 == 0:
                    nc.scalar.copy(o_sb[:, :], p1_ps[s][:, :])
                else:
                    nc.vector.tensor_copy(o_sb[:, :], p1_ps[s][:, :])
                nc.scalar.dma_start(out=y_h[s, :, :], in_=o_sb[:, :])

            # ---------- phase 2: sample pairs, shared-stationary 2-bank ----
            for pair in range(P1 // 2, S // 2):
                s0 = 2 * pair
                xp, mxp, psA, psB, rvp = [], [], [], [], []
                for j in range(2):
                    s = s0 + j
                    xp.append(load_x(s))
                    mxp.append(load_mx(s))
                    psA.append(pspool.tile([128, D_MODEL], f32, tag="ps",
                                           name=f"psA_{s}"))
                    psB.append(pspool.tile([128, D_MODEL], f32, tag="ps",
                                           name=f"psB_{s}"))
                    rvp.append(rv_of(s))
                for j in range(2):
                    rvA, rvB = rvp[j]
                    for k in range(N_SH):
                        # shared stationary x chunk, banks alternate A/B
                        nc.tensor.matmul(
                            psA[j][:, :],
                            xp[j][:, k * 128: (k + 1) * 128],
                            w_sb[k][:, bass.ds(rvA, D_MODEL)],
                            start=(k == 0), stop=False,
                        )
                        nc.tensor.matmul(
                            psB[j][:, :],
                            xp[j][:, k * 128: (k + 1) * 128],
                            w_sb[k][:, bass.ds(rvB, D_MODEL)],
                            start=(k == 0), stop=(k == N_SH - 1),
                        )
                # mixed blocks interleave the pair -> banks alternate
                for c in range(N_MIX):
                    for j in range(2):
                        mm_mix(psA[j], xp[j], mxp[j], c,
                               stop=(c == N_MIX - 1))
                for j in range(2):
                    s = s0 + j
                    t1 = tpool.tile([128, D_MODEL], f32, tag="t",
                                    name=f"t1_{s}")
                    nc.scalar.mul(t1[:, :], psB[j][:, :],
                                  g_sb[:, 2 * s + 1: 2 * s + 2])
                    o_sb = opool.tile([128, D_MODEL], bf16, tag="o",
                                      name=f"o_{s}")
                    # o = gA * psA + t1  (one fused DVE pass)
                    nc.vector.scalar_tensor_tensor(
                        o_sb[:, :], psA[j][:, :],
                        g_sb[:, 2 * s: 2 * s + 1], t1[:, :],
                        mybir.AluOpType.mult, mybir.AluOpType.add,
                    )
                    nc.scalar.dma_start(out=y_h[s, :, :], in_=o_sb[:, :])

    nc.finalize()
    return nc


def _gates_np(logits, moe_masks):
    """Mirror reference _gates in numpy (fp32)."""
    lg = logits.astype(np.float32)
    m = lg.max(axis=1, keepdims=True)
    e = np.exp(lg - m)
    g = e / e.sum(axis=1, keepdims=True)
    g = g * (moe_masks == 1).astype(np.float32)
    # top-2, ties -> lower index first (matches jax.lax.top_k)
    top_idx = np.argsort(-g, axis=1, kind="stable")[:, :TOP_K]
    rows = np.arange(g.shape[0])[:, None]
    gsel = g[rows, top_idx]                                  # [B, 2]
    gsel = gsel / (gsel.sum(axis=1, keepdims=True) + EPS)
    return gsel.astype(np.float32), top_idx.astype(np.int32)


def _prep_inputs(cycle_curve_data, logits, moe_masks, W, b):
    gsel, top_idx = _gates_np(logits, moe_masks)

    xf = cycle_curve_data.reshape(B, L, FEAT).astype(np.float32, copy=False)
    # x_host[s, p, k*128 + l] = x[s, l, k*128 + p] for chunks 0..6;
    # tail rows 896..899 + bias row at partitions 0..4, rest zero
    x_host = np.zeros((B, 128, XCOLS), np.float32)
    x_host[:, :, : 7 * 128] = np.ascontiguousarray(
        xf[:, :, : 7 * 128].reshape(B, L, 7, 128).transpose(0, 3, 2, 1)
    ).reshape(B, 128, 7 * 128)
    x_host[:, :4, 7 * 128:] = xf[:, :, 7 * 128: FEAT].transpose(0, 2, 1)
    x_host[:, 4, 7 * 128:] = 1.0                             # bias row
    x_host = x_host.astype(ml_dtypes.bfloat16)

    w_aug = np.concatenate(
        [W.astype(np.float32), b.astype(np.float32)[:, None, :]], axis=1
    )                                                        # [E, 901, 512]
    w_host = np.zeros((N_SH, 128, NUM_EXPERTS, D_MODEL), np.float32)
    for k in range(N_SH):
        w_host[k] = w_aug[:, k * 128: (k + 1) * 128, :].transpose(1, 0, 2)
    w_host = w_host.astype(ml_dtypes.bfloat16)

    # per-sample mixed weights for rows 640..900 (chunks 5, 6, tail)
    mix = (gsel[:, 0, None, None] * w_aug[top_idx[:, 0], MIX_ROW0:, :]
           + gsel[:, 1, None, None] * w_aug[top_idx[:, 1], MIX_ROW0:, :])
    # phase-1 samples (one-bank, gate-scaled x) take the raw mix;
    # phase-2 samples take mix/g0 so it can ride the expert-A bank
    phase1 = (np.arange(B) % S) < P1
    inv = np.where(phase1, 1.0,
                   1.0 / np.maximum(gsel[:, 0], 1e-9)).astype(np.float32)
    mix *= inv[:, None, None]
    mx_host = np.zeros((B, 128, MXCOLS), np.float32)
    mx_host[:, :, 0: D_MODEL] = mix[:, 0:128]                # chunk 5
    mx_host[:, :, D_MODEL: 2 * D_MODEL] = mix[:, 128:256]    # chunk 6
    mx_host[:, :K_LAST, 2 * D_MODEL:] = mix[:, 256:261]      # tail + bias
    mx_host = mx_host.astype(ml_dtypes.bfloat16)

    in_maps = []
    for c in range(N_CORES):
        sl = slice(c * S, (c + 1) * S)
        g_rep = np.broadcast_to(
            gsel[sl].reshape(1, 2 * S), (128, 2 * S)
        ).copy()
        widx = (top_idx[sl].reshape(1, 2 * S) * D_MODEL).astype(np.int32)
        in_maps.append({
            "x": np.ascontiguousarray(x_host[sl]),
            "w": w_host,
            "mx": np.ascontiguousarray(mx_host[sl]),
            "g": g_rep,
            "widx": widx,
        })
    return in_maps


def kernel(cycle_curve_data, logits, moe_masks, W, b):
    if "nc" not in _CACHE:
        _CACHE["nc"] = _build_nc()
    nc = _CACHE["nc"]

    in_maps = _prep_inputs(cycle_curve_data, logits, moe_masks, W, b)

    trace = bool(int(os.environ.get("KERNEL_PROFILE", "0")))
    res = run_bass_kernel_spmd(
        nc, in_maps, core_ids=list(range(N_CORES)), trace=trace
    )
    _CACHE["last_results"] = res

    out = np.empty((B, L, D_MODEL), ml_dtypes.bfloat16)
    for c in range(N_CORES):
        out[c * S: (c + 1) * S] = res.results[c]["y"]
    return out
